# revision 2
# baseline (speedup 1.0000x reference)
"""Trainium2 Bass kernel for nn_Attention_90220083019846.

Multi-head attention block: q/k/v = X@W{q,k,v}, scores = q@k^T + cb@k^T
(content bias folded into q), softmax, O = P@v, Z = X + O@Wo^T + b, LayerNorm.

Sharding over 8 NeuronCores: data-parallel over batch (2 groups of 4 cores) x
tensor-parallel over heads (4 heads per core). Output projection partial sums
are combined with half-query-block ReduceScatters within each batch group;
residual + LayerNorm run on the scattered shards.

Dataflow is fully "transposed": the host passes X^T, so every matmul contracts
over the partition axis with no on-device transposes. Projections and scores
run in bf16 (f32 PSUM accumulation) with full 128x128 stationary squares.
The probability matrix is fp8: exp runs on the scalar engine with a -12 bias
(softmax shift; normalization cancels it) writing e5m2 directly, and P@v runs
as fp8 DoubleRow matmuls (two 128-key planes per instruction) against an e4m3
v. Softmax sums ride along in the P@v matmul via a ones-column fused into v;
normalization happens on the small Oh tile (broadcast-matmul of the sums row
+ fast reciprocal). Head pairs share a 128-partition Oh tile (even head dims
on partitions 0-63, odd on 64-127, sums rows at 64/0) so the output
projection contracts fully dense 128-row stationaries.

The attention inner loop is software-pipelined at chain=(query block, head)
granularity: scores+exp of chain i issue before P@v+normalize of chain i-1,
so the PE never stalls at a chain boundary waiting for the last exp — the
scalar engine (the throughput limit at ~8us/chain) stays saturated. Stage A
is reordered (k, q[block0], v) so exp starts ~10us earlier, with the
remaining q blocks projected inside the chain loop. LayerNorm uses a
DVE-only rsqrt (magic-constant + Newton) to avoid ACT table thrashing with
exp, and is schedule-hinted behind the attention stream so the in-order DVE
queue never blocks compute behind a ReduceScatter wait.
"""

import contextlib
import ctypes
import sys
import types

sys.path.insert(0, "/opt/trn_rl_repo")

import numpy as np

# ---------------------------------------------------------------- profile hook
# The agent image's antenv lacks axon_hooks; provide it so that
# run_bass_kernel_spmd(trace=True) / BASS_TRACE=1 can capture NTFF profiles.
def _install_profile_hook():
    if "antenv.axon_hooks" in sys.modules:
        return
    try:
        import antenv
    except ImportError:
        return
    mod = types.ModuleType("antenv.axon_hooks")
    mod._hook = None
    mod.set_axon_ntff_profile_hook = lambda h: setattr(mod, "_hook", h)
    mod.get_axon_ntff_profile_hook = lambda: mod._hook
    sys.modules["antenv.axon_hooks"] = mod
    antenv.axon_hooks = mod
    try:
        lib = ctypes.CDLL("/opt/axon/libaxon_pjrt.so")
        if not hasattr(lib, "axon_start_nrt_profile"):
            return
        lib.axon_start_nrt_profile.argtypes = [
            ctypes.POINTER(ctypes.c_int64),
            ctypes.c_size_t,
        ]
        lib.axon_start_nrt_profile.restype = ctypes.c_int64
        lib.axon_stop_nrt_profile.argtypes = [ctypes.c_char_p]
        lib.axon_stop_nrt_profile.restype = ctypes.c_int64

        @contextlib.contextmanager
        def _hook(output_dir, device_ids):
            import jax

            jax.devices()
            if device_ids:
                ids = (ctypes.c_int64 * len(device_ids))(*device_ids)
                rc = lib.axon_start_nrt_profile(ids, len(device_ids))
            else:
                rc = lib.axon_start_nrt_profile(None, 0)
            if rc != 0:
                raise RuntimeError(f"axon_start_nrt_profile rc={rc}")
            try:
                yield
            finally:
                n = lib.axon_stop_nrt_profile(str(output_dir).encode())
                print(f"profile: {n} file(s) written to {output_dir}", file=sys.stderr)

        mod.set_axon_ntff_profile_hook(_hook)
    except OSError:
        pass


_install_profile_hook()

# ------------------------------------------------------------------- constants
B, L, D, H, HD = 2, 2048, 1024, 16, 64
NCORES = 8
GROUP = 4            # cores per batch group (tensor-parallel over heads)
HL = H // GROUP      # local heads per core
DL = HL * HD         # local head dims per core
QB = 512             # query block (tokens per pipeline chunk)
NQB = L // QB
RG = [[0, 1, 2, 3], [4, 5, 6, 7]]
LN_EPS = 1e-5
RSQRT_MAGIC = 0x5F3759DF
EXP_SHIFT = -12.0   # softmax shift: keeps e^score within fp8e5m2 range

_PROGRAM = None
LAST_RESULT = None


def _build_program():
    import concourse.tile as tile
    from concourse import bacc, mybir

    fr = mybir.dt.float32r
    f32 = mybir.dt.float32
    bf16 = mybir.dt.bfloat16
    i32 = mybir.dt.int32
    f8e4 = mybir.dt.float8e4
    f8e5 = mybir.dt.float8e5
    Exp = mybir.ActivationFunctionType.Exp
    Alu = mybir.AluOpType
    DRow = mybir.MatmulPerfMode.DoubleRow

    nc = bacc.Bacc("TRN2", target_bir_lowering=False, debug=False,
                   num_devices=NCORES)

    xt_d = nc.dram_tensor("xt", (D, L), bf16, kind="ExternalInput").ap()
    wq_d = nc.dram_tensor("wq", (D, DL), bf16, kind="ExternalInput").ap()
    wk_d = nc.dram_tensor("wk", (D, DL), bf16, kind="ExternalInput").ap()
    wv_d = nc.dram_tensor("wv", (D, DL), bf16, kind="ExternalInput").ap()
    wot_d = nc.dram_tensor("wot", (DL, D), bf16, kind="ExternalInput").ap()
    cb_d = nc.dram_tensor("cb", (DL, 1), f32, kind="ExternalInput").ap()
    xres_d = nc.dram_tensor("xres", (QB, D), f32, kind="ExternalInput").ap()
    lng_d = nc.dram_tensor("lng", (1, D), f32, kind="ExternalInput").ap()
    lnb_d = nc.dram_tensor("lnb", (1, D), f32, kind="ExternalInput").ap()
    ones_d = nc.dram_tensor("ones", (128, 2 * 128), fr, kind="ExternalInput").ap()
    out_d = nc.dram_tensor("out", (QB, D), f32, kind="ExternalOutput").ap()

    NH = 2 * NQB     # half-block ReduceScatter chunks
    ccin = [nc.dram_tensor(f"ccin{t}", (QB // 2, D), bf16, kind="Internal").ap()
            for t in range(NH)]
    ccout = [nc.dram_tensor(f"ccout{t}", (QB // 2 // GROUP, D), bf16,
                            kind="Internal").ap()
             for t in range(NH)]

    with tile.TileContext(nc) as tc, contextlib.ExitStack() as ctx:
        # ---------------- persistent pools
        wp = ctx.enter_context(tc.tile_pool(name="wp", bufs=1))
        kqv = ctx.enter_context(tc.tile_pool(name="kqv", bufs=1))
        cons = ctx.enter_context(tc.tile_pool(name="cons", bufs=1))
        # psp: scores / projection PSUM (3 banks x 2); oh and rb one bank each
        psp = ctx.enter_context(tc.tile_pool(name="psp", bufs=2, space="PSUM"))
        ohp = ctx.enter_context(tc.tile_pool(name="ohp", bufs=1, space="PSUM"))
        rbp = ctx.enter_context(tc.tile_pool(name="rbp", bufs=1, space="PSUM"))

        wq_t = wp.tile([128, 8, DL], bf16)
        wk_t = wp.tile([128, 8, DL], bf16)
        wv_t = wp.tile([128, 8, DL], bf16)
        wot_t = wp.tile([128, 2, D], bf16)   # dense head-pair chunks
        nc.sync.dma_start(out=wk_t, in_=wk_d.rearrange("(c p) o -> p c o", p=128))

        # k^T with the other head of the pair zeroed (full-square lhsT);
        # q^T keeps both heads (zero weights ignore the other head's rows)
        kt_ev = kqv.tile([128, 2, L], bf16)
        kt_od = kqv.tile([128, 2, L], bf16)
        qt = kqv.tile([128, 2, L], bf16)     # q^T (+cb)
        # fp8 v, augmented per 128-col head slot: even head = [v(64) | ones |
        # 63 zeros], odd head = [ones | 63 zeros | v(64)] — so a head pair's
        # Oh tiles stack into one 128-partition tile for the dense out-proj
        vaug = kqv.tile([128, L // 128, HL * 128], f8e4)
        ohn = kqv.tile([128, HL // 2, L], bf16)   # normalized Oh^T, head pairs
        xt = kqv.tile([128, 8, L], bf16)
        nc.gpsimd.memset(vaug, 0.0)
        vah = vaug.rearrange("p k (h x) -> p k h x", h=HL)
        for h in range(HL):
            col = HD if h % 2 == 0 else 0
            nc.vector.memset(vah[:, :, h, col:col + 1], 1.0)

        cb_t = cons.tile([128, 2], f32)
        nc.sync.dma_start(out=cb_t, in_=cb_d.rearrange("(m p) x -> p (m x)", p=128))
        # lhsT for the sums broadcast: ones_ev has row 64 set (even heads),
        # ones_od has row 0 set (odd heads); all else zero (f32r)
        ones_ev = cons.tile([128, 128], fr)
        ones_od = cons.tile([128, 128], fr)
        nc.sync.dma_start(out=ones_ev, in_=ones_d[:, 0:128])
        nc.sync.dma_start(out=ones_od, in_=ones_d[:, 128:256])
        lng_t = cons.tile([128, D], f32)
        nc.sync.dma_start(out=lng_t, in_=lng_d.partition_broadcast(128))
        lnb_t = cons.tile([128, D], f32)
        nc.sync.dma_start(out=lnb_t, in_=lnb_d.partition_broadcast(128))
        magic_t = cons.tile([128, 1], i32)
        nc.vector.memset(magic_t, RSQRT_MAGIC)
        shift_t = cons.tile([128, 1], f32)
        nc.vector.memset(shift_t, EXP_SHIFT)
        # row masks: keep one head of a pair, zero the other (f32 scalars)
        mask_lo = cons.tile([128, 1], f32)
        mask_hi = cons.tile([128, 1], f32)
        nc.vector.memset(mask_lo, 0.0)
        nc.vector.memset(mask_lo[0:64, :], 1.0)
        nc.vector.memset(mask_hi, 0.0)
        nc.vector.memset(mask_hi[64:128, :], 1.0)
        # residual rows (with Wo_b pre-folded) have no deps: prefetch all
        # four LN blocks' worth during stage A
        xr_t = []
        for qb in range(NQB):
            xr = cons.tile([128, D], f32)
            nc.sync.dma_start(out=xr, in_=xres_d[128 * qb:128 * (qb + 1), :])
            xr_t.append(xr)

        for t4 in range(NQB):
            for c in range(8):
                nc.sync.dma_start(
                    out=xt[:, c, QB * t4:QB * (t4 + 1)],
                    in_=xt_d[128 * c:128 * (c + 1), QB * t4:QB * (t4 + 1)])
        nc.sync.dma_start(out=wq_t, in_=wq_d.rearrange("(c p) o -> p c o", p=128))
        nc.sync.dma_start(out=wv_t, in_=wv_d.rearrange("(c p) o -> p c o", p=128))
        # dense 128-row chunks: rows pair with the packed head-pair ohn
        nc.sync.dma_start(out=wot_t,
                          in_=wot_d.rearrange("(c p) e -> p c e", p=128))

        # ---------------- stage A emitters (projections; contract over D)
        def emit_kq(w_t, t4, is_q):
            tsl = slice(QB * t4, QB * (t4 + 1))
            for m in range(2):
                ps = psp.tile([128, 3 * 512], f32, tag="ps")
                for c in range(8):
                    nc.tensor.matmul(
                        out=ps[:, 0:512],
                        lhsT=w_t[:, c, 128 * m:128 * (m + 1)],
                        rhs=xt[:, c, tsl],
                        start=(c == 0), stop=(c == 7),
                    )
                if is_q:
                    # ACT is idle until the first exp; offload the
                    # bias-add evacuation there
                    nc.scalar.add(out=qt[:, m, tsl], in_=ps[:, 0:512],
                                  add=cb_t[:, m:m + 1])
                else:
                    nc.vector.tensor_scalar_mul(
                        out=kt_ev[:, m, tsl], in0=ps[:, 0:512],
                        scalar1=mask_lo)
                    nc.vector.tensor_scalar_mul(
                        out=kt_od[:, m, tsl], in0=ps[:, 0:512],
                        scalar1=mask_hi)

        def emit_v(kc):
            ps = psp.tile([128, 3 * 512], f32, tag="ps")
            for c in range(8):
                nc.tensor.matmul(
                    out=ps[:, 0:DL],
                    lhsT=xt[:, c, 128 * kc:128 * (kc + 1)],
                    rhs=wv_t[:, c, :],
                    start=(c == 0), stop=(c == 7),
                )
            # split by head parity: even heads land at cols 0..63 of their
            # slot, odd heads at cols 64..127 (e indexes the head pair)
            pse = ps[:, 0:DL].rearrange("p (e f x) -> p e f x", e=2, f=2)
            va2 = vaug[:, kc, :].rearrange("p (e f x) -> p e f x", e=2, f=2)
            with nc.allow_low_precision(reason="fp8 v for DoubleRow P@v"):
                nc.vector.tensor_copy(out=va2[:, :, 0, 0:HD],
                                      in_=pse[:, :, 0, :])
                nc.vector.tensor_copy(out=va2[:, :, 1, HD:128],
                                      in_=pse[:, :, 1, :])

        with nc.named_scope("stageA"):
            for t4 in range(NQB):
                emit_kq(wk_t, t4, False)
            emit_kq(wq_t, 0, True)
            for kc in range(L // 128):
                emit_v(kc)

        # ---------------- stage B (attention) + stage C (proj/RS/LN),
        # software-pipelined one chain deep
        groups = [(0, 3), (3, 3), (6, 3), (9, 3), (12, 3), (15, 1)]
        nkc = L // 128

        with tc.tile_pool(name="ptp", bufs=3) as ptp, \
             tc.tile_pool(name="ohsp", bufs=2) as ohsp, \
             tc.tile_pool(name="recp", bufs=2) as recp, \
             tc.tile_pool(name="zevp", bufs=2) as zevp, \
             tc.tile_pool(name="lnp", bufs=2) as lnp:

            def emit_scores(qb, h):
                mi, par = h // 2, h % 2
                ktp = kt_ev if par == 0 else kt_od
                qT_b = qt[:, mi, QB * qb:QB * (qb + 1)]
                pt = ptp.tile([128, nkc, 512], f8e5, tag="pt")
                for kc0, n in groups:
                    st = psp.tile([128, 3 * 512], f32, tag="ps")
                    for i in range(n):
                        kc = kc0 + i
                        nc.tensor.matmul(
                            out=st[:, 512 * i:512 * (i + 1)],
                            lhsT=ktp[:, mi, 128 * kc:128 * (kc + 1)],
                            rhs=qT_b,
                            start=True, stop=True,
                        )
                    with nc.allow_low_precision(reason="fp8 softmax probs"):
                        nc.scalar.activation(
                            out=pt[:, kc0:kc0 + n, :],
                            in_=st[:, :512 * n].rearrange(
                                "p (n x) -> p n x", n=n),
                            func=Exp, bias=shift_t)
                return pt

            def emit_flush(qb, h, pt):
                mi, par = h // 2, h % 2
                oh = ohp.tile([128, 512], f32, tag="oh")
                # P@v: two 128-key fp8 planes per DoubleRow matmul
                for g in range(nkc // 2):
                    nc.tensor.matmul(
                        out=oh,
                        lhsT=vaug[:, 2 * g:2 * g + 2, 128 * h:128 * (h + 1)],
                        rhs=pt[:, 2 * g:2 * g + 2, :],
                        start=(g == 0), stop=(g == nkc // 2 - 1),
                        perf_mode=DRow,
                    )
                # evacuate Oh + sums, broadcast sums, reciprocal, scale.
                # even heads: dims on partitions 0-63, sums row at 64;
                # odd heads: sums row at 0, dims on partitions 64-127.
                dlo = 0 if par == 0 else 64
                ohs = ohsp.tile([128, 512], fr, tag="ohs")
                with nc.allow_low_precision(reason="f32r rounding of Oh"):
                    nc.vector.tensor_copy(out=ohs, in_=oh)
                rb = rbp.tile([128, 512], f32, tag="rb")
                nc.tensor.matmul(out=rb,
                                 lhsT=(ones_ev if par == 0 else ones_od)[0:65, :],
                                 rhs=ohs[0:65, :], start=True, stop=True)
                # full-128 at base partition 0: the custom DVE ucode anchors
                # at partition 0 and silently no-ops at higher bases
                rec = recp.tile([128, 512], f32, tag="rec")
                nc.vector.reciprocal_approx_fast(out=rec, in_=rb)
                nc.vector.tensor_mul(
                    out=ohn[dlo:dlo + 64, mi, QB * qb:QB * (qb + 1)],
                    in0=ohs[dlo:dlo + 64, :], in1=rec[dlo:dlo + 64, :])

            def emit_proj(qb):
                # output projection partials + half-block ReduceScatters
                for tcl in range(QB // 128):
                    t0 = QB * qb + 128 * tcl
                    zev = zevp.tile([128, D], bf16)
                    for ec in range(2):
                        zp = psp.tile([128, 3 * 512], f32, tag="ps")
                        for p2 in range(2):
                            nc.tensor.matmul(
                                out=zp[:, 0:512],
                                lhsT=ohn[:, p2, t0:t0 + 128],
                                rhs=wot_t[:, p2, 512 * ec:512 * (ec + 1)],
                                start=(p2 == 0), stop=(p2 == 1),
                            )
                        nc.vector.tensor_copy(out=zev[:, 512 * ec:512 * (ec + 1)],
                                              in_=zp[:, 0:512])
                    hc = 2 * qb + tcl // 2
                    nc.sync.dma_start(
                        out=ccin[hc][128 * (tcl % 2):128 * (tcl % 2 + 1), :],
                        in_=zev)
                    if tcl % 2 == 1:
                        nc.gpsimd.collective_compute(
                            "ReduceScatter", Alu.add,
                            ins=[ccin[hc][:]], outs=[ccout[hc][:]],
                            replica_groups=RG,
                        )

            chains = [(qb, h) for qb in range(NQB) for h in range(HL)]
            pend = None
            for qb, h in chains:
                with nc.named_scope(f"sc{qb}_{h}"):
                    pt = emit_scores(qb, h)
                if pend is not None:
                    pq, ph, ppt = pend
                    with nc.named_scope(f"fl{pq}_{ph}"):
                        emit_flush(pq, ph, ppt)
                    if ph == HL - 1:
                        with nc.named_scope(f"proj{pq}"):
                            emit_proj(pq)
                pend = (qb, h, pt)
                if qb == 0 and h > 0:
                    # project the next q block while attention streams
                    with nc.named_scope(f"qproj{h}"):
                        emit_kq(wq_t, h, True)
            pq, ph, ppt = pend
            with nc.named_scope("fl_last"):
                emit_flush(pq, ph, ppt)
            with nc.named_scope("proj_last"):
                emit_proj(NQB - 1)

            # ---- residual + LayerNorm (bias pre-folded into xres on host).
            # tile_wait_until holds each block back (absolute us!) so the
            # scheduler cannot hoist its DVE ops ahead of attention work,
            # where they would head-of-line-block the in-order DVE queue
            # waiting on the ReduceScatter.
            for qb in range(NQB):
              with tc.tile_wait_until(0.14 + 0.04 * qb), \
                   nc.named_scope(f"ln{qb}"):
                ccz = lnp.tile([128, D], bf16, tag="ccz")
                nc.sync.dma_start(out=ccz[0:64, :], in_=ccout[2 * qb])
                nc.sync.dma_start(out=ccz[64:128, :], in_=ccout[2 * qb + 1])
                zt = lnp.tile([128, D], f32, tag="zt")
                nc.vector.tensor_add(out=zt, in0=ccz, in1=xr_t[qb])

                stats = lnp.tile([128, 2, 6], f32, tag="stats")
                for sg in range(2):
                    nc.vector.bn_stats(out=stats[:, sg, :],
                                       in_=zt[:, 512 * sg:512 * (sg + 1)])
                mv = lnp.tile([128, 2], f32, tag="mv")
                nc.vector.bn_aggr(out=mv, in_=stats)

                # rstd = rsqrt(var + eps), DVE-only (avoids ACT table thrash)
                ve = lnp.tile([128, 1], f32, tag="ve")
                nc.vector.tensor_scalar_add(out=ve, in0=mv[:, 1:2], scalar1=LN_EPS)
                y = lnp.tile([128, 1], f32, tag="y")
                nc.vector.tensor_scalar(
                    out=y.bitcast(i32), in0=ve.bitcast(i32), scalar1=1,
                    scalar2=None, op0=Alu.logical_shift_right)
                nc.vector.tensor_sub(out=y.bitcast(i32), in0=magic_t,
                                     in1=y.bitcast(i32))
                tnw = lnp.tile([128, 1], f32, tag="tnw")
                for _ in range(3):
                    nc.vector.tensor_mul(out=tnw, in0=ve, in1=y)
                    nc.vector.tensor_mul(out=tnw, in0=tnw, in1=y)
                    nc.vector.tensor_scalar(out=tnw, in0=tnw, scalar1=-0.5,
                                            scalar2=1.5, op0=Alu.mult, op1=Alu.add)
                    nc.vector.tensor_mul(out=y, in0=y, in1=tnw)

                nc.vector.tensor_scalar(out=zt, in0=zt, scalar1=mv[:, 0:1],
                                        scalar2=y, op0=Alu.subtract, op1=Alu.mult)
                nc.vector.tensor_mul(out=zt, in0=zt, in1=lng_t)
                nc.vector.tensor_add(out=zt, in0=zt, in1=lnb_t)
                nc.sync.dma_start(out=out_d[128 * qb:128 * (qb + 1), :], in_=zt)


    nc.compile()
    return nc


def _get_program():
    global _PROGRAM
    if _PROGRAM is None:
        _PROGRAM = _build_program()
    return _PROGRAM


def _make_in_maps(X, Wq, Wk, Wv, cb, Wo_w, Wo_b, ln_g, ln_b):
    import ml_dtypes

    bf = ml_dtypes.bfloat16

    X = np.asarray(X, dtype=np.float32)
    Wq = np.asarray(Wq, dtype=np.float32)
    Wk = np.asarray(Wk, dtype=np.float32)
    Wv = np.asarray(Wv, dtype=np.float32)
    cb = np.asarray(cb, dtype=np.float32)
    Wo_w = np.asarray(Wo_w, dtype=np.float32)
    Wo_b = np.asarray(Wo_b, dtype=np.float32)
    ln_g = np.asarray(ln_g, dtype=np.float32)
    ln_b = np.asarray(ln_b, dtype=np.float32)

    WoT = np.ascontiguousarray(Wo_w.T)
    ones_arr = np.zeros((128, 2, 128), np.float32)
    ones_arr[64, 0, :] = 1.0   # even heads: sums row at partition 64
    ones_arr[0, 1, :] = 1.0    # odd heads: sums row at partition 0
    ones_arr = np.ascontiguousarray(ones_arr.reshape(128, 256))
    in_maps = []
    for c in range(NCORES):
        b, hp, r = c // GROUP, c % GROUP, c % GROUP
        Xb = X[b]
        rows = np.concatenate(
            [np.arange(QB * t + 256 * hh + 64 * r, QB * t + 256 * hh + 64 * r + 64)
             for t in range(NQB) for hh in range(2)])
        csl = slice(DL * hp, DL * (hp + 1))
        in_maps.append({
            "xt": np.ascontiguousarray(Xb.T).astype(bf),
            "xres": np.ascontiguousarray(Xb[rows] + Wo_b),
            "wq": np.ascontiguousarray(Wq[:, csl]).astype(bf),
            "wk": np.ascontiguousarray(Wk[:, csl]).astype(bf),
            "wv": np.ascontiguousarray(Wv[:, csl]).astype(bf),
            "wot": np.ascontiguousarray(WoT[csl, :]).astype(bf),
            "cb": np.ascontiguousarray(cb[csl].reshape(DL, 1)),
            "lng": np.ascontiguousarray(ln_g.reshape(1, D)),
            "lnb": np.ascontiguousarray(ln_b.reshape(1, D)),
            "ones": ones_arr,
        })
    return in_maps


def _gather_out(results):
    out = np.empty((B, L, D), np.float32)
    for cid in range(NCORES):
        b, r = cid // GROUP, cid % GROUP
        o = results[cid]["out"]
        for t in range(NQB):
            for hh in range(2):
                g0 = QB * t + 256 * hh + 64 * r
                out[b, g0:g0 + 64] = o[128 * t + 64 * hh:128 * t + 64 * (hh + 1)]
    return out


def kernel(X, Y, Wq, Wk, Wv, cb, Wo_w, Wo_b, ln_g, ln_b):
    from concourse import bass_utils

    prog = _get_program()
    in_maps = _make_in_maps(X, Wq, Wk, Wv, cb, Wo_w, Wo_b, ln_g, ln_b)

    res = bass_utils.run_bass_kernel_spmd(prog, in_maps, core_ids=list(range(NCORES)))
    global LAST_RESULT
    LAST_RESULT = res

    return _gather_out(res.results)


if __name__ == "__main__":
    rng = np.random.default_rng(0)
    ins = {
        "X": rng.standard_normal((B, L, D)).astype(np.float32),
        "Y": rng.standard_normal((B, L, D)).astype(np.float32),
        "Wq": (rng.uniform(-1, 1, (D, D)) / 32).astype(np.float32),
        "Wk": (rng.uniform(-1, 1, (D, D)) / 32).astype(np.float32),
        "Wv": (rng.uniform(-1, 1, (D, D)) / 32).astype(np.float32),
        "cb": np.zeros(D, np.float32),
        "Wo_w": (rng.uniform(-1, 1, (D, D)) / 32).astype(np.float32),
        "Wo_b": (rng.uniform(-1, 1, D) / 32).astype(np.float32),
        "ln_g": np.ones(D, np.float32),
        "ln_b": np.zeros(D, np.float32),
    }
    out = kernel(**ins)
    print("out", out.shape, out.dtype, float(np.abs(out).max()))
    print("exec_time_ns:", LAST_RESULT.exec_time_ns)
